# revision 55
# baseline (speedup 1.0000x reference)
"""Trainium2 Bass kernel for per-bag softmax attention pooling (v2, fp16).

Problem: x [100000, 768] f32, attention_query [100000, 3] int, scope =
arange(12501)*8 (uniform bags of 8 consecutive sentences), attention_matrix
[130, 768] f32.

    att = attention_matrix[attention_query]          # [N, 3, 768]
    logits = einsum('nd,nld->nl', x, att)            # [N, 3]
    w = softmax(logits within each bag of 8)         # [N, 3]
    out[l, b, :] = sum_{n in bag b} w[n, l] * x[n]   # [3, 12500, 768]

Data-parallel over bags: 8 cores x 1568 bags (12544 sentences) each, padded
with zero bags from 12500 to 12544 total.

v3 design (probe- and sim-driven; v1 was DMA-bound with a partition-
starved output store, v2 was ACT/DVE-bound on per-instruction overhead):
  - x ships to DRAM as fp16 (host cast): input traffic halved. rel-err
    budget 2e-2 is ~40x above the observed fp16 error (5e-4).
  - output is written TRANSPOSED: p2[p, 48j + 16l + b] =
    out[l, bag, 128j+p], i.e. [128, 288] fp16 tiles -> all 128 DMA
    partitions active (v1 stored [48, 768] f32: 48 partitions = ~6/16
    SDMA engines and 4x the bytes).
  - all matmuls are fp16 (1 cycle/row; fp32r at free-dim < 256 runs at
    4 cycles/row when warm, which made v1's y-matmuls 4x slower).
  - softmax weights are pre-normalized on the fly: per-bag sums come from
    one tiny matmul against a constant 128x128 block-diagonal bag-
    adjacency matrix (s_sent = bagadj @ e), then w = e * recip(s) before
    the weighted-sum matmul. This kills v1's 768-col normalize pass.
  - tiles are processed in PAIRS and the output store runs on the idle
    GpSimd queue: ACT/DVE instruction fixed costs (~130-250ns each)
    dominate; pairing halves the per-tile instruction count of every
    small op (exp, bag-sum matmul, reciprocal, normalize, y-copy,
    output copy, store dispatch).

Per tile-pair (2x 128 sentences = 32 bags):
  A : 12 PE transposes (fp16) -> PSUM, cast-copy to xts fp16 (DVE/ACT).
  B1: y = xT.T @ A.T  [128, 2x130] via 12 fp16 matmuls; one ACT f16
      copy of y to SBUF.
  B1x (one iteration later): 6 fused DVE scalar_tensor_tensor ops
      extract logit_l = y[n, q_l] into one [128, 8] tile — deferring
      this breaks the per-iteration PE->ACT->DVE tail chain.
  B2a: e8 = exp(logits) (one ACT op); s_sent = bagadj @ e8 (one PE MM,
      emitted last in PE program order to avoid a mid-queue stall).
  B2b: en = e8 * recip(s_sent) (DVE); wb[n, 48k + 16l + b].
  C1: 12 fp16 matmuls into p2 [128, 800] (tile1 at f32 col 512 so no
      matmul output straddles a PSUM bank).
  C2: two cast-copies (DVE cols 0:288, ACT 512:800) -> [128, 576] fp16,
      one 1152B/partition DMA store per pair on the GpSimd queue.
"""

import json
import os

import numpy as np

import concourse.bass as bass
import concourse.mybir as mybir
from concourse.bass_utils import run_bass_kernel_spmd
from concourse.tile import TileContext

# ---------------------------------------------------------------------------
# walrus codegen in this container accepts only ONE sync-wait command per
# instruction (CTRL, S3_LW, ... structs), but Tile's add_sem_waits freely
# attaches one wait per producer proc. Post-process the serialized BIR:
# hoist excess waits onto standalone EventSemaphore instructions (the same
# thing bass's wait_ge emits) inserted right before the offender, on the
# same engine.
# ---------------------------------------------------------------------------
_orig_to_json_bytes = getattr(
    bass.Bass.to_json_bytes, "_split_waits_orig", bass.Bass.to_json_bytes
)


def _to_json_bytes_split_waits(self, *args, **kwargs):
    raw = _orig_to_json_bytes(self, *args, **kwargs)
    bir = json.loads(raw)
    ctr = 0
    changed = False
    for fn in bir.get("functions", []):
        for bb in fn.get("blocks", []):
            insts = bb.get("instructions", [])
            out = []
            for inst in insts:
                si = inst.get("sync_info")
                ow = (si or {}).get("on_wait") or []
                if len(ow) > 1:
                    changed = True
                    for w in ow[:-1]:
                        ctr += 1
                        out.append(
                            {
                                "debug": inst.get("debug"),
                                "engine": inst["engine"],
                                "ins": [],
                                "name": f"I-splitw{ctr}",
                                "opcode": "EventSemaphore",
                                "outs": [],
                                "sync_info": {"on_update": [], "on_wait": [w]},
                            }
                        )
                    si["on_wait"] = [ow[-1]]
                out.append(inst)
            bb["instructions"] = out
    if not changed:
        return raw
    return json.dumps(bir).encode()


_to_json_bytes_split_waits._split_waits_orig = _orig_to_json_bytes
bass.Bass.to_json_bytes = _to_json_bytes_split_waits

# ---------------------------------------------------------------------------
# Problem constants (hardcoded; kernel.py must be self-contained).
# ---------------------------------------------------------------------------
N = 100000          # sentences
D = 768             # relation dim
G = 130             # classes
SEG = 8             # sentences per bag
B = N // SEG        # 12500 bags
NCORES = 8
P = 128             # partitions / sentences per tile
BAGS_PER_TILE = P // SEG            # 16
DCHUNKS = D // P                    # 6
NT = 98                             # tiles per core
ROWS_CORE = NT * P                  # 12544 sentences per core
BAGS_CORE = ROWS_CORE // SEG        # 1568 bags per core
N_PAD = ROWS_CORE * NCORES          # 100352
F16 = mybir.dt.float16
F32 = mybir.dt.float32
F32R = mybir.dt.float32r
NB = BAGS_PER_TILE      # 16
NL3 = 3 * NB            # 48
OUTW = DCHUNKS * NL3    # 288

LAST_EXEC_TIME_NS = None
LAST_TRACE_PATH = None


def build_nc(n_tiles=NT, passes=1):
    nc = bass.Bass("TRN2", target_bir_lowering=False)

    x_in = nc.dram_tensor("x", [n_tiles * P, D], F16, kind="ExternalInput")
    q_in = nc.dram_tensor("q", [P, 3 * n_tiles], F32, kind="ExternalInput")
    at_in = nc.dram_tensor("at", [P, G * DCHUNKS], F16, kind="ExternalInput")
    id_in = nc.dram_tensor("ident", [P, P], F16, kind="ExternalInput")
    iota_in = nc.dram_tensor("iota", [P, G], F16, kind="ExternalInput")
    mask_in = nc.dram_tensor("mask16", [P, NB], F16, kind="ExternalInput")
    adj_in = nc.dram_tensor("bagadj", [P, P], F16, kind="ExternalInput")
    # output stores are batched one tile-pair per DMA: 576B/partition
    # descriptors hit the SDMA small-transfer penalty, 1152B do not.
    assert n_tiles % 2 == 0
    out = nc.dram_tensor(
        "out", [(n_tiles // 2) * P, 2 * OUTW], F16, kind="ExternalOutput"
    )

    eq = mybir.AluOpType.is_equal
    mult = mybir.AluOpType.mult
    div = mybir.AluOpType.divide
    dbl = os.environ.get("KERNEL_DOUBLE", "")
    SPLIT = int(os.environ.get("KERNEL_SPLIT", "448"))    # xts copy DVE share

    with TileContext(nc) as tc:
        with (
            tc.tile_pool(name="const", bufs=1) as cpool,
            tc.tile_pool(name="sbuf", bufs=3) as pool,
            tc.tile_pool(name="sbxz", bufs=9) as pxz,
            tc.tile_pool(name="sbxts", bufs=3) as pxts,
            tc.tile_pool(name="sbmid", bufs=3) as pmid,
            tc.tile_pool(name="sbout", bufs=3) as pouts,
            tc.tile_pool(name="ps1", bufs=1, space="PSUM") as ppool1,
            tc.tile_pool(name="ps2", bufs=2, space="PSUM") as ppool2,
            tc.tile_pool(name="ps3", bufs=2, space="PSUM") as ppool3,
        ):
            id_sb = cpool.tile([P, P], F16, tag="ident")
            nc.sync.dma_start(id_sb[:, :], id_in[:, :])
            at_sb = cpool.tile([P, G * DCHUNKS], F16, tag="at")
            nc.sync.dma_start(at_sb[:, :], at_in[:, :])
            iota_sb = cpool.tile([P, G], F16, tag="iota")
            nc.sync.dma_start(iota_sb[:, :], iota_in[:, :])
            mask_sb = cpool.tile([P, NB], F16, tag="mask16")
            nc.sync.dma_start(mask_sb[:, :], mask_in[:, :])
            adj_sb = cpool.tile([P, P], F16, tag="bagadj")
            nc.sync.dma_start(adj_sb[:, :], adj_in[:, :])
            q_sb = cpool.tile([P, 3 * n_tiles], F32, tag="q")
            nc.sync.dma_start(q_sb[:, :], q_in[:, :])

            # Tiles are processed in PAIRS: every small fixed-cost op (exp,
            # bag-sum matmul, reciprocal, normalize, y-copy, output copy,
            # store) runs once per two tiles — per-instruction overhead
            # (~200-250ns on ACT) dominated the v2 critical path.
            #
            # logit tiles live in a manual ring: tile0 logits in cols 0:3,
            # tile1 in 4:7; cols 3 and 7 are zeroed once and never
            # rewritten, so exp() can process [128, 8] wholesale (those
            # lanes keep e=1, feeding unused s_sent columns).
            NRING = 3
            logit_ring = []
            for k in range(NRING):
                lt = cpool.tile([P, 8], F32, tag=f"logit{k}")
                nc.vector.memset(lt[:, 3:4], 0.0)
                nc.vector.memset(lt[:, 7:8], 0.0)
                logit_ring.append(lt)
            # p2 pair layout: tile0 output at cols 0:288, tile1 at 512:800
            # (f32 col 512 = byte 2048 = PSUM bank boundary; no matmul
            # output may straddle a bank).
            T1OFF = 512

            def stageL(g):
                # both tiles of pair g land in one SBUF buffer, loaded by
                # two contiguous-block DMAs (a single 2-segment-AP DMA
                # measured slower on HW).
                xzp = pxz.tile([P, 2 * D], F16, tag="xz")
                for k in (0, 1):
                    t = 2 * g + k
                    nc.sync.dma_start(
                        xzp[:, k * D : (k + 1) * D],
                        x_in[t * P : (t + 1) * P, :],
                    )
                    if dbl == "DMAIN":
                        nc.sync.dma_start(
                            xzp[:, k * D : (k + 1) * D],
                            x_in[t * P : (t + 1) * P, :],
                        )
                return xzp

            def stageA(g, xzp):
                xtp = ppool1.tile([P, 2 * D], F16, tag="xtp")
                for rr in range(2 if dbl == "T" else 1):
                    for c in range(12):
                        nc.tensor.transpose(
                            xtp[:, c * P : (c + 1) * P],
                            xzp[:, c * P : (c + 1) * P],
                            id_sb[:, :],
                        )
                xts = pxts.tile([P, 2 * D], F16, tag="xts")
                for rr in range(2 if dbl == "COPY" else 1):
                    if SPLIT > 0:
                        nc.vector.tensor_copy(xts[:, 0:SPLIT], xtp[:, 0:SPLIT])
                    if SPLIT < 2 * D:
                        nc.scalar.copy(
                            xts[:, SPLIT : 2 * D], xtp[:, SPLIT : 2 * D]
                        )
                return xts

            def stageB1(g, xts):
                # [0:260] = both tiles' y; [264:272] hosts a later pair's
                # ss result (sadj writes into the freshest yp buffer) so ss
                # needs no PSUM bank of its own -> p2 can double-buffer.
                yp = ppool2.tile([P, 272], F32, tag="yp")
                nreps = 2 if dbl == "Y" else 1
                for k in (0, 1):
                    for rr in range(nreps):
                        for j in range(6):
                            nc.tensor.matmul(
                                yp[:, k * G : (k + 1) * G],
                                xts[:, k * D + j * P : k * D + (j + 1) * P],
                                at_sb[:, j * G : (j + 1) * G],
                                start=(rr == 0 and j == 0),
                                stop=(rr == nreps - 1 and j == 5),
                            )
                # f16 SBUF copy of both tiles' y: the extraction ops then
                # run all-16-bit on SBUF operands (DVE 2x mode).
                ysb = pool.tile([P, 2 * G], F16, tag="ysb")
                for rr in range(2 if dbl == "YSB" else 1):
                    nc.scalar.copy(ysb[:, :], yp[:, 0 : 2 * G])
                return ysb, yp

            def stageB1x(g, ysb):
                # runs one iteration after B1 so the DVE extraction never
                # waits on this iteration's y-matmuls -> ySB chain
                logit = logit_ring[g % NRING]
                scratch = pool.tile([P, G], F16, tag="scratch")
                for rr in range(2 if dbl == "STT" else 1):
                    for k in (0, 1):
                        for layer in range(3):
                            nc.vector.scalar_tensor_tensor(
                                scratch[:, :],
                                iota_sb[:, :],
                                q_sb[
                                    :,
                                    3 * (2 * g + k) + layer :
                                    3 * (2 * g + k) + layer + 1,
                                ],
                                ysb[:, k * G : (k + 1) * G],
                                op0=eq,
                                op1=mult,
                                accum_out=logit[:, 4 * k + layer : 4 * k + layer + 1],
                            )
                return logit

            def stageB2a_exp(g, logit):
                e8 = pool.tile([P, 8], F16, tag="e8")
                nc.scalar.activation(
                    e8[:, :], logit[:, :], mybir.ActivationFunctionType.Exp
                )
                return e8

            def stageB2a_sadj(g, e8, host_yp):
                # emitted LAST in PE program order so the wait on exp(g)
                # lands after this iteration's transposes, not mid-stream
                ss = host_yp[:, 264:272]
                nc.tensor.matmul(
                    ss, adj_sb[:, :], e8[:, :], start=True, stop=True
                )
                return ss

            def stageB2b(g, e8, ss):
                rs = pool.tile([P, 8], F32, tag="rs")
                nc.vector.reciprocal(rs[:, :], ss)
                en = pool.tile([P, 8], F32, tag="en")
                nc.vector.tensor_mul(en[:, :], e8[:, :], rs[:, :])
                # single broadcast tensor_tensor builds all 96 weight cols:
                # wb[:, 48k + 16l + b] = mask16[:, b] * en[:, 4k + l]
                # (stride-0 dims broadcast mask over (k,l) and en over b)
                wb = pmid.tile([P, 2 * NL3], F16, tag="wb")
                mask_b = bass.AP(
                    mask_sb[:, :].tensor, mask_sb[:, :].offset,
                    [[NB, P], [0, 2], [0, 3], [1, NB]],
                )
                en_b = bass.AP(
                    en[:, :].tensor, en[:, :].offset,
                    [[8, P], [4, 2], [1, 3], [0, NB]],
                )
                wb_b = bass.AP(
                    wb[:, :].tensor, wb[:, :].offset,
                    [[2 * NL3, P], [NL3, 2], [NB, 3], [1, NB]],
                )
                for rr in range(2 if dbl == "WB" else 1):
                    nc.vector.tensor_tensor(wb_b, mask_b, en_b, op=mult)
                return wb

            def stageC1(g, xzp, wb):
                p2 = ppool3.tile([P, T1OFF + OUTW], F32, tag="p2")
                nreps = 2 if dbl == "P2" else 1
                for rr in range(nreps):
                    for k in (0, 1):
                        off = k * T1OFF
                        for j in range(6):
                            nc.tensor.matmul(
                                p2[:, off + j * NL3 : off + (j + 1) * NL3],
                                xzp[:, k * D + j * P : k * D + (j + 1) * P],
                                wb[:, k * NL3 : (k + 1) * NL3],
                                start=(rr == 0),
                                stop=(rr == nreps - 1),
                            )
                return p2

            odma = {
                "pool": nc.gpsimd,
                "sync": nc.sync,
                "act": nc.scalar,
            }[os.environ.get("KERNEL_ODMA", "pool")]

            def stageC2(g, p2):
                outs = pouts.tile([P, 2 * OUTW], F16, tag="outs")
                for rr in range(2 if dbl == "C2" else 1):
                    nc.vector.tensor_copy(outs[:, 0:OUTW], p2[:, 0:OUTW])
                    nc.scalar.copy(
                        outs[:, OUTW : 2 * OUTW],
                        p2[:, T1OFF : T1OFF + OUTW],
                    )
                for rr in range(2 if dbl == "DMAOUT" else 1):
                    odma.dma_start(out[g * P : (g + 1) * P, :], outs[:, :])

            NP = n_tiles // 2
            for rep in range(passes):
                stL = {}
                stXts = {}
                stLog = {}
                stE = {}
                stSS = {}
                stWb = {}
                stP2 = {}
                stYsb = {}
                for i in range(-2, NP + 7):
                    if 0 <= i + 2 < NP:
                        stL[i + 2] = stageL(i + 2)
                    if 0 <= i - 6 < NP:
                        stageC2(i - 6, stP2.pop(i - 6))
                    if 0 <= i - 5 < NP:
                        stP2[i - 5] = stageC1(
                            i - 5, stL.pop(i - 5), stWb.pop(i - 5)
                        )
                    if 0 <= i - 4 < NP:
                        stWb[i - 4] = stageB2b(
                            i - 4, stE.pop(i - 4), stSS.pop(i - 4)
                        )
                    if 0 <= i - 3 < NP:
                        stE[i - 3] = stageB2a_exp(i - 3, stLog.pop(i - 3))
                    if 0 <= i - 2 < NP:
                        stLog[i - 2] = stageB1x(i - 2, stYsb.pop(i - 2))
                    if 0 <= i - 1 < NP:
                        stYsb[i - 1], last_yp = stageB1(
                            i - 1, stXts.pop(i - 1)
                        )
                    if 0 <= i < NP:
                        stXts[i] = stageA(i, stL[i])
                    if 0 <= i - 3 < NP:
                        stSS[i - 3] = stageB2a_sadj(
                            i - 3, stE[i - 3], last_yp
                        )

    return nc


# ---------------------------------------------------------------------------
# Host-side constants + sharding
# ---------------------------------------------------------------------------


def _host_constants(attention_matrix):
    a = np.ascontiguousarray(np.asarray(attention_matrix, dtype=np.float32))
    assert a.shape == (G, D)
    at = a.T  # [768, 130]
    at_r = np.ascontiguousarray(
        at.reshape(DCHUNKS, P, G).transpose(1, 0, 2).reshape(P, DCHUNKS * G)
    ).astype(np.float16)
    ident = np.eye(P, dtype=np.float16)
    iota = np.tile(np.arange(G, dtype=np.float16), (P, 1))
    mask16 = (
        (np.arange(P)[:, None] // SEG) == np.arange(NB)[None, :]
    ).astype(np.float16)
    bagadj = (
        (np.arange(P)[:, None] // SEG) == (np.arange(P)[None, :] // SEG)
    ).astype(np.float16)
    return at_r, ident, iota, mask16, bagadj


def make_in_maps(x, attention_query, attention_matrix):
    at_r, ident, iota, mask16, bagadj = _host_constants(attention_matrix)

    x_pad = np.zeros((N_PAD, D), dtype=np.float16)
    x_pad[:N] = x.astype(np.float16)
    q_pad = np.zeros((N_PAD, 3), dtype=np.float32)
    q_pad[:N] = attention_query.astype(np.float32)

    in_maps = []
    for c in range(NCORES):
        xs = x_pad[c * ROWS_CORE : (c + 1) * ROWS_CORE]
        qs = (
            q_pad[c * ROWS_CORE : (c + 1) * ROWS_CORE]
            .reshape(NT, P, 3)
            .transpose(1, 0, 2)
            .reshape(P, 3 * NT)
        )
        in_maps.append(
            {
                "x": np.ascontiguousarray(xs),
                "q": np.ascontiguousarray(qs),
                "at": at_r,
                "ident": ident,
                "iota": iota,
                "mask16": mask16,
                "bagadj": bagadj,
            }
        )
    return in_maps


def kernel(x, attention_query, scope, attention_matrix):
    x = np.asarray(x)
    attention_query = np.asarray(attention_query)
    assert x.shape == (N, D) and attention_query.shape == (N, 3)

    in_maps = make_in_maps(x, attention_query, attention_matrix)

    nc = build_nc()
    trace = bool(int(os.environ.get("KERNEL_TRACE", "0")))
    res = run_bass_kernel_spmd(
        nc, in_maps, core_ids=list(range(NCORES)), trace=trace
    )
    global LAST_EXEC_TIME_NS, LAST_TRACE_PATH
    LAST_EXEC_TIME_NS = res.exec_time_ns
    if trace:
        print(f"HW exec time: {res.exec_time_ns} ns")
        if res.instructions_and_trace is not None:
            LAST_TRACE_PATH = res.instructions_and_trace[1]
            print("trace:", LAST_TRACE_PATH)

    # per-core out is [(NT/2)*128, 2*288] fp16: pair g, partition p, col
    # k*288 + 48j + 16l + b = out[l, 16(2g+k) + b, 128j + p]
    parts = [
        r["out"]
        .reshape(NT // 2, P, 2, DCHUNKS, 3, NB)
        .transpose(4, 0, 2, 5, 3, 1)
        .reshape(3, BAGS_CORE, D)
        for r in res.results
    ]
    full = np.concatenate(parts, axis=1)[:, :B, :].astype(np.float32)
    return np.ascontiguousarray(full)


# revision 57
# speedup vs baseline: 1.4610x; 1.4610x over previous
"""Trainium2 Bass kernel for per-bag softmax attention pooling (v2, fp16).

Problem: x [100000, 768] f32, attention_query [100000, 3] int, scope =
arange(12501)*8 (uniform bags of 8 consecutive sentences), attention_matrix
[130, 768] f32.

    att = attention_matrix[attention_query]          # [N, 3, 768]
    logits = einsum('nd,nld->nl', x, att)            # [N, 3]
    w = softmax(logits within each bag of 8)         # [N, 3]
    out[l, b, :] = sum_{n in bag b} w[n, l] * x[n]   # [3, 12500, 768]

Data-parallel over bags: 8 cores x 1568 bags (12544 sentences) each, padded
with zero bags from 12500 to 12544 total.

v3 design (probe- and sim-driven; v1 was DMA-bound with a partition-
starved output store, v2 was ACT/DVE-bound on per-instruction overhead):
  - x ships to DRAM as fp16 (host cast): input traffic halved. rel-err
    budget 2e-2 is ~40x above the observed fp16 error (5e-4).
  - output is written TRANSPOSED: p2[p, 48j + 16l + b] =
    out[l, bag, 128j+p], i.e. [128, 288] fp16 tiles -> all 128 DMA
    partitions active (v1 stored [48, 768] f32: 48 partitions = ~6/16
    SDMA engines and 4x the bytes).
  - all matmuls are fp16 (1 cycle/row; fp32r at free-dim < 256 runs at
    4 cycles/row when warm, which made v1's y-matmuls 4x slower).
  - softmax weights are pre-normalized on the fly: per-bag sums come from
    one tiny matmul against a constant 128x128 block-diagonal bag-
    adjacency matrix (s_sent = bagadj @ e), then w = e * recip(s) before
    the weighted-sum matmul. This kills v1's 768-col normalize pass.
  - tiles are processed in PAIRS and the output store runs on the idle
    GpSimd queue: ACT/DVE instruction fixed costs (~130-250ns each)
    dominate; pairing halves the per-tile instruction count of every
    small op (exp, bag-sum matmul, reciprocal, normalize, y-copy,
    output copy, store dispatch).

Per tile-pair (2x 128 sentences = 32 bags):
  A : 12 PE transposes (fp16) -> PSUM, cast-copy to xts fp16 (DVE/ACT).
  B1: y = xT.T @ A.T  [128, 2x130] via 12 fp16 matmuls; one ACT f16
      copy of y to SBUF.
  B1x (one iteration later): 6 fused DVE scalar_tensor_tensor ops
      extract logit_l = y[n, q_l] into one [128, 8] tile — deferring
      this breaks the per-iteration PE->ACT->DVE tail chain.
  B2a: e8 = exp(logits) (one ACT op); s_sent = bagadj @ e8 (one PE MM,
      emitted last in PE program order to avoid a mid-queue stall).
  B2b: en = e8 * recip(s_sent) (DVE); wb[n, 48k + 16l + b].
  C1: 12 fp16 matmuls into p2 [128, 800] (tile1 at f32 col 512 so no
      matmul output straddles a PSUM bank).
  C2: two cast-copies (DVE cols 0:288, ACT 512:800) -> [128, 576] fp16,
      one 1152B/partition DMA store per pair on the GpSimd queue.
"""

import json
import os

import numpy as np

import concourse.bass as bass
import concourse.mybir as mybir
from concourse.bass_utils import run_bass_kernel_spmd
from concourse.tile import TileContext

# ---------------------------------------------------------------------------
# walrus codegen in this container accepts only ONE sync-wait command per
# instruction (CTRL, S3_LW, ... structs), but Tile's add_sem_waits freely
# attaches one wait per producer proc. Post-process the serialized BIR:
# hoist excess waits onto standalone EventSemaphore instructions (the same
# thing bass's wait_ge emits) inserted right before the offender, on the
# same engine.
# ---------------------------------------------------------------------------
_orig_to_json_bytes = getattr(
    bass.Bass.to_json_bytes, "_split_waits_orig", bass.Bass.to_json_bytes
)


def _to_json_bytes_split_waits(self, *args, **kwargs):
    raw = _orig_to_json_bytes(self, *args, **kwargs)
    bir = json.loads(raw)
    ctr = 0
    changed = False
    for fn in bir.get("functions", []):
        for bb in fn.get("blocks", []):
            insts = bb.get("instructions", [])
            out = []
            for inst in insts:
                si = inst.get("sync_info")
                ow = (si or {}).get("on_wait") or []
                if len(ow) > 1:
                    changed = True
                    for w in ow[:-1]:
                        ctr += 1
                        out.append(
                            {
                                "debug": inst.get("debug"),
                                "engine": inst["engine"],
                                "ins": [],
                                "name": f"I-splitw{ctr}",
                                "opcode": "EventSemaphore",
                                "outs": [],
                                "sync_info": {"on_update": [], "on_wait": [w]},
                            }
                        )
                    si["on_wait"] = [ow[-1]]
                out.append(inst)
            bb["instructions"] = out
    if not changed:
        return raw
    return json.dumps(bir).encode()


_to_json_bytes_split_waits._split_waits_orig = _orig_to_json_bytes
bass.Bass.to_json_bytes = _to_json_bytes_split_waits

# ---------------------------------------------------------------------------
# Problem constants (hardcoded; kernel.py must be self-contained).
# ---------------------------------------------------------------------------
N = 100000          # sentences
D = 768             # relation dim
G = 130             # classes
SEG = 8             # sentences per bag
B = N // SEG        # 12500 bags
NCORES = 8
P = 128             # partitions / sentences per tile
BAGS_PER_TILE = P // SEG            # 16
DCHUNKS = D // P                    # 6
NT = 98                             # tiles per core
ROWS_CORE = NT * P                  # 12544 sentences per core
BAGS_CORE = ROWS_CORE // SEG        # 1568 bags per core
N_PAD = ROWS_CORE * NCORES          # 100352
F16 = mybir.dt.float16
F32 = mybir.dt.float32
F32R = mybir.dt.float32r
NB = BAGS_PER_TILE      # 16
NL3 = 3 * NB            # 48
OUTW = DCHUNKS * NL3    # 288

LAST_EXEC_TIME_NS = None
LAST_TRACE_PATH = None


def build_nc(n_tiles=NT, passes=1):
    nc = bass.Bass("TRN2", target_bir_lowering=False)

    x_in = nc.dram_tensor("x", [n_tiles * P, D], F16, kind="ExternalInput")
    q_in = nc.dram_tensor("q", [P, 3 * n_tiles], F32, kind="ExternalInput")
    at_in = nc.dram_tensor("at", [P, G * DCHUNKS], F16, kind="ExternalInput")
    id_in = nc.dram_tensor("ident", [P, P], F16, kind="ExternalInput")
    iota_in = nc.dram_tensor("iota", [P, G], F16, kind="ExternalInput")
    mask_in = nc.dram_tensor("mask16", [P, NB], F16, kind="ExternalInput")
    adj_in = nc.dram_tensor("bagadj", [P, P], F16, kind="ExternalInput")
    # output stores are batched one tile-pair per DMA: 576B/partition
    # descriptors hit the SDMA small-transfer penalty, 1152B do not.
    assert n_tiles % 2 == 0
    out = nc.dram_tensor(
        "out", [(n_tiles // 2) * P, 2 * OUTW], F16, kind="ExternalOutput"
    )

    eq = mybir.AluOpType.is_equal
    mult = mybir.AluOpType.mult
    div = mybir.AluOpType.divide
    dbl = os.environ.get("KERNEL_DOUBLE", "")
    SPLIT = int(os.environ.get("KERNEL_SPLIT", "448"))    # xts copy DVE share

    with TileContext(nc) as tc:
        with (
            tc.tile_pool(name="const", bufs=1) as cpool,
            tc.tile_pool(name="sbuf", bufs=3) as pool,
            tc.tile_pool(name="sbxz", bufs=9) as pxz,
            tc.tile_pool(name="sbxts", bufs=3) as pxts,
            tc.tile_pool(name="sbmid", bufs=3) as pmid,
            tc.tile_pool(name="sbout", bufs=3) as pouts,
            tc.tile_pool(name="ps1", bufs=1, space="PSUM") as ppool1,
            tc.tile_pool(name="ps2", bufs=2, space="PSUM") as ppool2,
            tc.tile_pool(name="pss", bufs=2, space="PSUM") as ppool_s,
            tc.tile_pool(name="ps3", bufs=1, space="PSUM") as ppool3,
        ):
            id_sb = cpool.tile([P, P], F16, tag="ident")
            nc.sync.dma_start(id_sb[:, :], id_in[:, :])
            at_sb = cpool.tile([P, G * DCHUNKS], F16, tag="at")
            nc.sync.dma_start(at_sb[:, :], at_in[:, :])
            iota_sb = cpool.tile([P, G], F16, tag="iota")
            nc.sync.dma_start(iota_sb[:, :], iota_in[:, :])
            mask_sb = cpool.tile([P, NB], F16, tag="mask16")
            nc.sync.dma_start(mask_sb[:, :], mask_in[:, :])
            adj_sb = cpool.tile([P, P], F16, tag="bagadj")
            nc.sync.dma_start(adj_sb[:, :], adj_in[:, :])
            q_sb = cpool.tile([P, 3 * n_tiles], F32, tag="q")
            nc.sync.dma_start(q_sb[:, :], q_in[:, :])

            # Tiles are processed in PAIRS: every small fixed-cost op (exp,
            # bag-sum matmul, reciprocal, normalize, y-copy, output copy,
            # store) runs once per two tiles — per-instruction overhead
            # (~200-250ns on ACT) dominated the v2 critical path.
            #
            # logit tiles live in a manual ring: tile0 logits in cols 0:3,
            # tile1 in 4:7; cols 3 and 7 are zeroed once and never
            # rewritten, so exp() can process [128, 8] wholesale (those
            # lanes keep e=1, feeding unused s_sent columns).
            NRING = 3
            logit_ring = []
            for k in range(NRING):
                lt = cpool.tile([P, 8], F32, tag=f"logit{k}")
                nc.vector.memset(lt[:, 3:4], 0.0)
                nc.vector.memset(lt[:, 7:8], 0.0)
                logit_ring.append(lt)
            # p2 pair layout: tile0 output at cols 0:288, tile1 at 512:800
            # (f32 col 512 = byte 2048 = PSUM bank boundary; no matmul
            # output may straddle a bank).
            T1OFF = 512

            def stageL(g):
                # both tiles of pair g land in one SBUF buffer, loaded by
                # two contiguous-block DMAs (a single 2-segment-AP DMA
                # measured slower on HW).
                xzp = pxz.tile([P, 2 * D], F16, tag="xz")
                for k in (0, 1):
                    t = 2 * g + k
                    nc.sync.dma_start(
                        xzp[:, k * D : (k + 1) * D],
                        x_in[t * P : (t + 1) * P, :],
                    )
                    if dbl == "DMAIN":
                        nc.sync.dma_start(
                            xzp[:, k * D : (k + 1) * D],
                            x_in[t * P : (t + 1) * P, :],
                        )
                return xzp

            def stageAt(g, xzp):
                # transposes are emitted FIRST in each iteration: their only
                # input (xz, loaded 2 iterations ago) is always ready, so PE
                # has guaranteed work while DVE/ACT drain the C2 copies that
                # this iteration's C1 waits on (p2 is single-buffered).
                xtp = ppool1.tile([P, 2 * D], F16, tag="xtp")
                for rr in range(2 if dbl == "T" else 1):
                    for c in range(12):
                        nc.tensor.transpose(
                            xtp[:, c * P : (c + 1) * P],
                            xzp[:, c * P : (c + 1) * P],
                            id_sb[:, :],
                        )
                return xtp

            def stageAc(g, xtp):
                # emitted at the DVE/ACT queue tails: by then this
                # iteration's transposes are long done
                xts = pxts.tile([P, 2 * D], F16, tag="xts")
                for rr in range(2 if dbl == "COPY" else 1):
                    if SPLIT > 0:
                        nc.vector.tensor_copy(xts[:, 0:SPLIT], xtp[:, 0:SPLIT])
                    if SPLIT < 2 * D:
                        nc.scalar.copy(
                            xts[:, SPLIT : 2 * D], xtp[:, SPLIT : 2 * D]
                        )
                return xts

            def stageB1(g, xts):
                yp = ppool2.tile([P, 2 * G], F32, tag="yp")
                nreps = 2 if dbl == "Y" else 1
                for k in (0, 1):
                    for rr in range(nreps):
                        for j in range(6):
                            nc.tensor.matmul(
                                yp[:, k * G : (k + 1) * G],
                                xts[:, k * D + j * P : k * D + (j + 1) * P],
                                at_sb[:, j * G : (j + 1) * G],
                                start=(rr == 0 and j == 0),
                                stop=(rr == nreps - 1 and j == 5),
                            )
                return yp

            def stageB1c(g, yp):
                # f16 SBUF copy of both tiles' y: the extraction ops then
                # run all-16-bit on SBUF operands (DVE 2x mode). Emitted
                # after the xts ACT copy so ACT never idles waiting on
                # this iteration's y-matmuls.
                ysb = pool.tile([P, 2 * G], F16, tag="ysb")
                for rr in range(2 if dbl == "YSB" else 1):
                    nc.scalar.copy(ysb[:, :], yp[:, :])
                return ysb

            def stageB1x(g, ysb):
                # runs one iteration after B1 so the DVE extraction never
                # waits on this iteration's y-matmuls -> ySB chain
                logit = logit_ring[g % NRING]
                scratch = pool.tile([P, G], F16, tag="scratch")
                for rr in range(2 if dbl == "STT" else 1):
                    for k in (0, 1):
                        for layer in range(3):
                            nc.vector.scalar_tensor_tensor(
                                scratch[:, :],
                                iota_sb[:, :],
                                q_sb[
                                    :,
                                    3 * (2 * g + k) + layer :
                                    3 * (2 * g + k) + layer + 1,
                                ],
                                ysb[:, k * G : (k + 1) * G],
                                op0=eq,
                                op1=mult,
                                accum_out=logit[:, 4 * k + layer : 4 * k + layer + 1],
                            )
                return logit

            def stageB2a_exp(g, logit):
                e8 = pool.tile([P, 8], F16, tag="e8")
                nc.scalar.activation(
                    e8[:, :], logit[:, :], mybir.ActivationFunctionType.Exp
                )
                return e8

            def stageB2a_sadj(g, e8):
                # emitted LAST in PE program order so the wait on exp(g)
                # lands after this iteration's transposes, not mid-stream
                ss = ppool_s.tile([P, 8], F32, tag="ss")
                nc.tensor.matmul(
                    ss[:, :], adj_sb[:, :], e8[:, :], start=True, stop=True
                )
                return ss

            def stageB2b(g, e8, ss):
                rs = pool.tile([P, 8], F32, tag="rs")
                nc.vector.reciprocal(rs[:, :], ss[:, :])
                en = pool.tile([P, 8], F32, tag="en")
                nc.vector.tensor_mul(en[:, :], e8[:, :], rs[:, :])
                # single broadcast tensor_tensor builds all 96 weight cols:
                # wb[:, 48k + 16l + b] = mask16[:, b] * en[:, 4k + l]
                # (stride-0 dims broadcast mask over (k,l) and en over b)
                wb = pmid.tile([P, 2 * NL3], F16, tag="wb")
                mask_b = bass.AP(
                    mask_sb[:, :].tensor, mask_sb[:, :].offset,
                    [[NB, P], [0, 2], [0, 3], [1, NB]],
                )
                en_b = bass.AP(
                    en[:, :].tensor, en[:, :].offset,
                    [[8, P], [4, 2], [1, 3], [0, NB]],
                )
                wb_b = bass.AP(
                    wb[:, :].tensor, wb[:, :].offset,
                    [[2 * NL3, P], [NL3, 2], [NB, 3], [1, NB]],
                )
                for rr in range(2 if dbl == "WB" else 1):
                    nc.vector.tensor_tensor(wb_b, mask_b, en_b, op=mult)
                return wb

            def stageC1(g, xzp, wb):
                p2 = ppool3.tile([P, T1OFF + OUTW], F32, tag="p2")
                nreps = 2 if dbl == "P2" else 1
                for rr in range(nreps):
                    for k in (0, 1):
                        off = k * T1OFF
                        for j in range(6):
                            nc.tensor.matmul(
                                p2[:, off + j * NL3 : off + (j + 1) * NL3],
                                xzp[:, k * D + j * P : k * D + (j + 1) * P],
                                wb[:, k * NL3 : (k + 1) * NL3],
                                start=(rr == 0),
                                stop=(rr == nreps - 1),
                            )
                return p2

            odma = {
                "pool": nc.gpsimd,
                "sync": nc.sync,
                "act": nc.scalar,
            }[os.environ.get("KERNEL_ODMA", "pool")]

            def stageC2(g, p2):
                outs = pouts.tile([P, 2 * OUTW], F16, tag="outs")
                for rr in range(2 if dbl == "C2" else 1):
                    nc.vector.tensor_copy(outs[:, 0:OUTW], p2[:, 0:OUTW])
                    nc.scalar.copy(
                        outs[:, OUTW : 2 * OUTW],
                        p2[:, T1OFF : T1OFF + OUTW],
                    )
                for rr in range(2 if dbl == "DMAOUT" else 1):
                    odma.dma_start(out[g * P : (g + 1) * P, :], outs[:, :])

            NP = n_tiles // 2
            for rep in range(passes):
                stL = {}
                stXts = {}
                stLog = {}
                stE = {}
                stSS = {}
                stWb = {}
                stP2 = {}
                stYsb = {}
                stXtp = {}
                stYp = {}
                for i in range(-2, NP + 7):
                    if 0 <= i < NP:
                        stXtp[i] = stageAt(i, stL[i])
                    if 0 <= i + 2 < NP:
                        stL[i + 2] = stageL(i + 2)
                    if 0 <= i - 6 < NP:
                        stageC2(i - 6, stP2.pop(i - 6))
                    if 0 <= i - 5 < NP:
                        stP2[i - 5] = stageC1(
                            i - 5, stL.pop(i - 5), stWb.pop(i - 5)
                        )
                    if 0 <= i - 4 < NP:
                        stWb[i - 4] = stageB2b(
                            i - 4, stE.pop(i - 4), stSS.pop(i - 4)
                        )
                    if 0 <= i - 3 < NP:
                        stE[i - 3] = stageB2a_exp(i - 3, stLog.pop(i - 3))
                    if 0 <= i - 2 < NP:
                        stLog[i - 2] = stageB1x(i - 2, stYsb.pop(i - 2))
                    if 0 <= i - 1 < NP:
                        stYp[i - 1] = stageB1(i - 1, stXts.pop(i - 1))
                    if 0 <= i < NP:
                        stXts[i] = stageAc(i, stXtp.pop(i))
                    if 0 <= i - 1 < NP:
                        stYsb[i - 1] = stageB1c(i - 1, stYp.pop(i - 1))
                    if 0 <= i - 3 < NP:
                        stSS[i - 3] = stageB2a_sadj(i - 3, stE[i - 3])

    return nc


# ---------------------------------------------------------------------------
# Host-side constants + sharding
# ---------------------------------------------------------------------------


def _host_constants(attention_matrix):
    a = np.ascontiguousarray(np.asarray(attention_matrix, dtype=np.float32))
    assert a.shape == (G, D)
    at = a.T  # [768, 130]
    at_r = np.ascontiguousarray(
        at.reshape(DCHUNKS, P, G).transpose(1, 0, 2).reshape(P, DCHUNKS * G)
    ).astype(np.float16)
    ident = np.eye(P, dtype=np.float16)
    iota = np.tile(np.arange(G, dtype=np.float16), (P, 1))
    mask16 = (
        (np.arange(P)[:, None] // SEG) == np.arange(NB)[None, :]
    ).astype(np.float16)
    bagadj = (
        (np.arange(P)[:, None] // SEG) == (np.arange(P)[None, :] // SEG)
    ).astype(np.float16)
    return at_r, ident, iota, mask16, bagadj


def make_in_maps(x, attention_query, attention_matrix):
    at_r, ident, iota, mask16, bagadj = _host_constants(attention_matrix)

    x_pad = np.zeros((N_PAD, D), dtype=np.float16)
    x_pad[:N] = x.astype(np.float16)
    q_pad = np.zeros((N_PAD, 3), dtype=np.float32)
    q_pad[:N] = attention_query.astype(np.float32)

    in_maps = []
    for c in range(NCORES):
        xs = x_pad[c * ROWS_CORE : (c + 1) * ROWS_CORE]
        qs = (
            q_pad[c * ROWS_CORE : (c + 1) * ROWS_CORE]
            .reshape(NT, P, 3)
            .transpose(1, 0, 2)
            .reshape(P, 3 * NT)
        )
        in_maps.append(
            {
                "x": np.ascontiguousarray(xs),
                "q": np.ascontiguousarray(qs),
                "at": at_r,
                "ident": ident,
                "iota": iota,
                "mask16": mask16,
                "bagadj": bagadj,
            }
        )
    return in_maps


def kernel(x, attention_query, scope, attention_matrix):
    x = np.asarray(x)
    attention_query = np.asarray(attention_query)
    assert x.shape == (N, D) and attention_query.shape == (N, 3)

    in_maps = make_in_maps(x, attention_query, attention_matrix)

    nc = build_nc()
    trace = bool(int(os.environ.get("KERNEL_TRACE", "0")))
    res = run_bass_kernel_spmd(
        nc, in_maps, core_ids=list(range(NCORES)), trace=trace
    )
    global LAST_EXEC_TIME_NS, LAST_TRACE_PATH
    LAST_EXEC_TIME_NS = res.exec_time_ns
    if trace:
        print(f"HW exec time: {res.exec_time_ns} ns")
        if res.instructions_and_trace is not None:
            LAST_TRACE_PATH = res.instructions_and_trace[1]
            print("trace:", LAST_TRACE_PATH)

    # per-core out is [(NT/2)*128, 2*288] fp16: pair g, partition p, col
    # k*288 + 48j + 16l + b = out[l, 16(2g+k) + b, 128j + p]
    parts = [
        r["out"]
        .reshape(NT // 2, P, 2, DCHUNKS, 3, NB)
        .transpose(4, 0, 2, 5, 3, 1)
        .reshape(3, BAGS_CORE, D)
        for r in res.results
    ]
    full = np.concatenate(parts, axis=1)[:, :B, :].astype(np.float32)
    return np.ascontiguousarray(full)


# revision 58
# speedup vs baseline: 2.0735x; 1.4193x over previous
"""Trainium2 Bass kernel for per-bag softmax attention pooling (v2, fp16).

Problem: x [100000, 768] f32, attention_query [100000, 3] int, scope =
arange(12501)*8 (uniform bags of 8 consecutive sentences), attention_matrix
[130, 768] f32.

    att = attention_matrix[attention_query]          # [N, 3, 768]
    logits = einsum('nd,nld->nl', x, att)            # [N, 3]
    w = softmax(logits within each bag of 8)         # [N, 3]
    out[l, b, :] = sum_{n in bag b} w[n, l] * x[n]   # [3, 12500, 768]

Data-parallel over bags: 8 cores x 1568 bags (12544 sentences) each, padded
with zero bags from 12500 to 12544 total.

v3 design (probe- and sim-driven; v1 was DMA-bound with a partition-
starved output store, v2 was ACT/DVE-bound on per-instruction overhead):
  - x ships to DRAM as fp16 (host cast): input traffic halved. rel-err
    budget 2e-2 is ~40x above the observed fp16 error (5e-4).
  - output is written TRANSPOSED: p2[p, 48j + 16l + b] =
    out[l, bag, 128j+p], i.e. [128, 288] fp16 tiles -> all 128 DMA
    partitions active (v1 stored [48, 768] f32: 48 partitions = ~6/16
    SDMA engines and 4x the bytes).
  - all matmuls are fp16 (1 cycle/row; fp32r at free-dim < 256 runs at
    4 cycles/row when warm, which made v1's y-matmuls 4x slower).
  - softmax weights are pre-normalized on the fly: per-bag sums come from
    one tiny matmul against a constant 128x128 block-diagonal bag-
    adjacency matrix (s_sent = bagadj @ e), then w = e * recip(s) before
    the weighted-sum matmul. This kills v1's 768-col normalize pass.
  - tiles are processed in PAIRS and the output store runs on the idle
    GpSimd queue: ACT/DVE instruction fixed costs (~130-250ns each)
    dominate; pairing halves the per-tile instruction count of every
    small op (exp, bag-sum matmul, reciprocal, normalize, y-copy,
    output copy, store dispatch).

Per tile-pair (2x 128 sentences = 32 bags):
  A : 12 PE transposes (fp16) -> PSUM, cast-copy to xts fp16 (DVE/ACT).
  B1: y = xT.T @ A.T  [128, 2x130] via 12 fp16 matmuls; one ACT f16
      copy of y to SBUF.
  B1x (one iteration later): 6 fused DVE scalar_tensor_tensor ops
      extract logit_l = y[n, q_l] into one [128, 8] tile — deferring
      this breaks the per-iteration PE->ACT->DVE tail chain.
  B2a: e8 = exp(logits) (one ACT op); s_sent = bagadj @ e8 (one PE MM,
      emitted last in PE program order to avoid a mid-queue stall).
  B2b: en = e8 * recip(s_sent) (DVE); wb[n, 48k + 16l + b].
  C1: 12 fp16 matmuls into p2 [128, 800] (tile1 at f32 col 512 so no
      matmul output straddles a PSUM bank).
  C2: two cast-copies (DVE cols 0:288, ACT 512:800) -> [128, 576] fp16,
      one 1152B/partition DMA store per pair on the GpSimd queue.
"""

import json
import os

import numpy as np

import concourse.bass as bass
import concourse.mybir as mybir
from concourse.bass_utils import run_bass_kernel_spmd
from concourse.tile import TileContext

# ---------------------------------------------------------------------------
# walrus codegen in this container accepts only ONE sync-wait command per
# instruction (CTRL, S3_LW, ... structs), but Tile's add_sem_waits freely
# attaches one wait per producer proc. Post-process the serialized BIR:
# hoist excess waits onto standalone EventSemaphore instructions (the same
# thing bass's wait_ge emits) inserted right before the offender, on the
# same engine.
# ---------------------------------------------------------------------------
_orig_to_json_bytes = getattr(
    bass.Bass.to_json_bytes, "_split_waits_orig", bass.Bass.to_json_bytes
)


def _to_json_bytes_split_waits(self, *args, **kwargs):
    raw = _orig_to_json_bytes(self, *args, **kwargs)
    bir = json.loads(raw)
    ctr = 0
    changed = False
    for fn in bir.get("functions", []):
        for bb in fn.get("blocks", []):
            insts = bb.get("instructions", [])
            out = []
            for inst in insts:
                si = inst.get("sync_info")
                ow = (si or {}).get("on_wait") or []
                if len(ow) > 1:
                    changed = True
                    for w in ow[:-1]:
                        ctr += 1
                        out.append(
                            {
                                "debug": inst.get("debug"),
                                "engine": inst["engine"],
                                "ins": [],
                                "name": f"I-splitw{ctr}",
                                "opcode": "EventSemaphore",
                                "outs": [],
                                "sync_info": {"on_update": [], "on_wait": [w]},
                            }
                        )
                    si["on_wait"] = [ow[-1]]
                out.append(inst)
            bb["instructions"] = out
    if not changed:
        return raw
    return json.dumps(bir).encode()


_to_json_bytes_split_waits._split_waits_orig = _orig_to_json_bytes
bass.Bass.to_json_bytes = _to_json_bytes_split_waits

# ---------------------------------------------------------------------------
# Problem constants (hardcoded; kernel.py must be self-contained).
# ---------------------------------------------------------------------------
N = 100000          # sentences
D = 768             # relation dim
G = 130             # classes
SEG = 8             # sentences per bag
B = N // SEG        # 12500 bags
NCORES = 8
P = 128             # partitions / sentences per tile
BAGS_PER_TILE = P // SEG            # 16
DCHUNKS = D // P                    # 6
NT = 98                             # tiles per core
ROWS_CORE = NT * P                  # 12544 sentences per core
BAGS_CORE = ROWS_CORE // SEG        # 1568 bags per core
N_PAD = ROWS_CORE * NCORES          # 100352
F16 = mybir.dt.float16
F32 = mybir.dt.float32
F32R = mybir.dt.float32r
NB = BAGS_PER_TILE      # 16
NL3 = 3 * NB            # 48
OUTW = DCHUNKS * NL3    # 288

LAST_EXEC_TIME_NS = None
LAST_TRACE_PATH = None


def build_nc(n_tiles=NT, passes=1):
    nc = bass.Bass("TRN2", target_bir_lowering=False)

    x_in = nc.dram_tensor("x", [n_tiles * P, D], F16, kind="ExternalInput")
    q_in = nc.dram_tensor("q", [P, 3 * n_tiles], F32, kind="ExternalInput")
    at_in = nc.dram_tensor("at", [P, G * DCHUNKS], F16, kind="ExternalInput")
    id_in = nc.dram_tensor("ident", [P, P], F16, kind="ExternalInput")
    iota_in = nc.dram_tensor("iota", [P, G], F16, kind="ExternalInput")
    mask_in = nc.dram_tensor("mask16", [P, NB], F16, kind="ExternalInput")
    adj_in = nc.dram_tensor("bagadj", [P, P], F16, kind="ExternalInput")
    # output stores are batched one tile-pair per DMA: 576B/partition
    # descriptors hit the SDMA small-transfer penalty, 1152B do not.
    assert n_tiles % 2 == 0
    out = nc.dram_tensor(
        "out", [(n_tiles // 2) * P, 2 * OUTW], F16, kind="ExternalOutput"
    )

    eq = mybir.AluOpType.is_equal
    mult = mybir.AluOpType.mult
    div = mybir.AluOpType.divide
    dbl = os.environ.get("KERNEL_DOUBLE", "")
    SPLIT = int(os.environ.get("KERNEL_SPLIT", "448"))    # xts copy DVE share

    with TileContext(nc) as tc:
        with (
            tc.tile_pool(name="const", bufs=1) as cpool,
            tc.tile_pool(name="sbuf", bufs=3) as pool,
            tc.tile_pool(name="sbxz", bufs=9) as pxz,
            tc.tile_pool(name="sbxts", bufs=3) as pxts,
            tc.tile_pool(name="sbmid", bufs=3) as pmid,
            tc.tile_pool(name="sbout", bufs=3) as pouts,
            tc.tile_pool(name="ps1", bufs=1, space="PSUM") as ppool1,
            tc.tile_pool(name="ps2", bufs=1, space="PSUM") as ppool2,
            tc.tile_pool(name="pss", bufs=1, space="PSUM") as ppool_s,
            tc.tile_pool(name="ps3", bufs=2, space="PSUM") as ppool3,
        ):
            id_sb = cpool.tile([P, P], F16, tag="ident")
            nc.sync.dma_start(id_sb[:, :], id_in[:, :])
            at_sb = cpool.tile([P, G * DCHUNKS], F16, tag="at")
            nc.sync.dma_start(at_sb[:, :], at_in[:, :])
            iota_sb = cpool.tile([P, G], F16, tag="iota")
            nc.sync.dma_start(iota_sb[:, :], iota_in[:, :])
            mask_sb = cpool.tile([P, NB], F16, tag="mask16")
            nc.sync.dma_start(mask_sb[:, :], mask_in[:, :])
            adj_sb = cpool.tile([P, P], F16, tag="bagadj")
            nc.sync.dma_start(adj_sb[:, :], adj_in[:, :])
            q_sb = cpool.tile([P, 3 * n_tiles], F32, tag="q")
            nc.sync.dma_start(q_sb[:, :], q_in[:, :])

            # Tiles are processed in PAIRS: every small fixed-cost op (exp,
            # bag-sum matmul, reciprocal, normalize, y-copy, output copy,
            # store) runs once per two tiles — per-instruction overhead
            # (~200-250ns on ACT) dominated the v2 critical path.
            #
            # logit tiles live in a manual ring: tile0 logits in cols 0:3,
            # tile1 in 4:7; cols 3 and 7 are zeroed once and never
            # rewritten, so exp() can process [128, 8] wholesale (those
            # lanes keep e=1, feeding unused s_sent columns).
            NRING = 3
            logit_ring = []
            for k in range(NRING):
                lt = cpool.tile([P, 8], F32, tag=f"logit{k}")
                nc.vector.memset(lt[:, 3:4], 0.0)
                nc.vector.memset(lt[:, 7:8], 0.0)
                logit_ring.append(lt)
            # p2 pair layout: tile0 output at cols 0:288, tile1 at 512:800
            # (f32 col 512 = byte 2048 = PSUM bank boundary; no matmul
            # output may straddle a bank).
            T1OFF = 512

            def stageL(g):
                # both tiles of pair g land in one SBUF buffer, loaded by
                # two contiguous-block DMAs (a single 2-segment-AP DMA
                # measured slower on HW).
                xzp = pxz.tile([P, 2 * D], F16, tag="xz")
                for k in (0, 1):
                    t = 2 * g + k
                    nc.sync.dma_start(
                        xzp[:, k * D : (k + 1) * D],
                        x_in[t * P : (t + 1) * P, :],
                    )
                    if dbl == "DMAIN":
                        nc.sync.dma_start(
                            xzp[:, k * D : (k + 1) * D],
                            x_in[t * P : (t + 1) * P, :],
                        )
                return xzp

            def stageAt(g, xzp):
                # transposes are emitted FIRST in each iteration: their only
                # input (xz, loaded 2 iterations ago) is always ready, so PE
                # has guaranteed work while DVE/ACT drain the C2 copies that
                # this iteration's C1 waits on (p2 is single-buffered).
                xtp = ppool1.tile([P, 2 * D], F16, tag="xtp")
                for rr in range(2 if dbl == "T" else 1):
                    for c in range(12):
                        nc.tensor.transpose(
                            xtp[:, c * P : (c + 1) * P],
                            xzp[:, c * P : (c + 1) * P],
                            id_sb[:, :],
                        )
                return xtp

            def stageAc(g, xtp):
                # emitted at the DVE/ACT queue tails: by then this
                # iteration's transposes are long done
                xts = pxts.tile([P, 2 * D], F16, tag="xts")
                for rr in range(2 if dbl == "COPY" else 1):
                    if SPLIT > 0:
                        nc.vector.tensor_copy(xts[:, 0:SPLIT], xtp[:, 0:SPLIT])
                    if SPLIT < 2 * D:
                        nc.scalar.copy(
                            xts[:, SPLIT : 2 * D], xtp[:, SPLIT : 2 * D]
                        )
                return xts

            def stageB1(g, xts):
                yp = ppool2.tile([P, 2 * G], F32, tag="yp")
                nreps = 2 if dbl == "Y" else 1
                for k in (0, 1):
                    for rr in range(nreps):
                        for j in range(6):
                            nc.tensor.matmul(
                                yp[:, k * G : (k + 1) * G],
                                xts[:, k * D + j * P : k * D + (j + 1) * P],
                                at_sb[:, j * G : (j + 1) * G],
                                start=(rr == 0 and j == 0),
                                stop=(rr == nreps - 1 and j == 5),
                            )
                return yp

            def stageB1c(g, yp):
                # f16 SBUF copy of both tiles' y: the extraction ops then
                # run all-16-bit on SBUF operands (DVE 2x mode). Emitted
                # after the xts ACT copy so ACT never idles waiting on
                # this iteration's y-matmuls.
                ysb = pool.tile([P, 2 * G], F16, tag="ysb")
                for rr in range(2 if dbl == "YSB" else 1):
                    nc.scalar.copy(ysb[:, :], yp[:, :])
                return ysb

            def stageB1x(g, ysb):
                # runs one iteration after B1 so the DVE extraction never
                # waits on this iteration's y-matmuls -> ySB chain
                logit = logit_ring[g % NRING]
                scratch = pool.tile([P, G], F16, tag="scratch")
                for rr in range(2 if dbl == "STT" else 1):
                    for k in (0, 1):
                        for layer in range(3):
                            nc.vector.scalar_tensor_tensor(
                                scratch[:, :],
                                iota_sb[:, :],
                                q_sb[
                                    :,
                                    3 * (2 * g + k) + layer :
                                    3 * (2 * g + k) + layer + 1,
                                ],
                                ysb[:, k * G : (k + 1) * G],
                                op0=eq,
                                op1=mult,
                                accum_out=logit[:, 4 * k + layer : 4 * k + layer + 1],
                            )
                return logit

            def stageB2a_exp(g, logit):
                e8 = pool.tile([P, 8], F16, tag="e8")
                nc.scalar.activation(
                    e8[:, :], logit[:, :], mybir.ActivationFunctionType.Exp
                )
                return e8

            def stageB2a_sadj(g, e8):
                # emitted LAST in PE program order so the wait on exp(g)
                # lands after this iteration's transposes, not mid-stream
                ss = ppool_s.tile([P, 8], F32, tag="ss")
                nc.tensor.matmul(
                    ss[:, :], adj_sb[:, :], e8[:, :], start=True, stop=True
                )
                return ss

            def stageB2b(g, e8, ss):
                rs = pool.tile([P, 8], F32, tag="rs")
                nc.vector.reciprocal(rs[:, :], ss[:, :])
                en = pool.tile([P, 8], F32, tag="en")
                nc.vector.tensor_mul(en[:, :], e8[:, :], rs[:, :])
                # single broadcast tensor_tensor builds all 96 weight cols:
                # wb[:, 48k + 16l + b] = mask16[:, b] * en[:, 4k + l]
                # (stride-0 dims broadcast mask over (k,l) and en over b)
                wb = pmid.tile([P, 2 * NL3], F16, tag="wb")
                mask_b = bass.AP(
                    mask_sb[:, :].tensor, mask_sb[:, :].offset,
                    [[NB, P], [0, 2], [0, 3], [1, NB]],
                )
                en_b = bass.AP(
                    en[:, :].tensor, en[:, :].offset,
                    [[8, P], [4, 2], [1, 3], [0, NB]],
                )
                wb_b = bass.AP(
                    wb[:, :].tensor, wb[:, :].offset,
                    [[2 * NL3, P], [NL3, 2], [NB, 3], [1, NB]],
                )
                for rr in range(2 if dbl == "WB" else 1):
                    nc.vector.tensor_tensor(wb_b, mask_b, en_b, op=mult)
                return wb

            def stageC1(g, xzp, wb):
                p2 = ppool3.tile([P, T1OFF + OUTW], F32, tag="p2")
                nreps = 2 if dbl == "P2" else 1
                for rr in range(nreps):
                    for k in (0, 1):
                        off = k * T1OFF
                        for j in range(6):
                            nc.tensor.matmul(
                                p2[:, off + j * NL3 : off + (j + 1) * NL3],
                                xzp[:, k * D + j * P : k * D + (j + 1) * P],
                                wb[:, k * NL3 : (k + 1) * NL3],
                                start=(rr == 0),
                                stop=(rr == nreps - 1),
                            )
                return p2

            odma = {
                "pool": nc.gpsimd,
                "sync": nc.sync,
                "act": nc.scalar,
            }[os.environ.get("KERNEL_ODMA", "pool")]

            def stageC2(g, p2):
                outs = pouts.tile([P, 2 * OUTW], F16, tag="outs")
                for rr in range(2 if dbl == "C2" else 1):
                    nc.vector.tensor_copy(outs[:, 0:OUTW], p2[:, 0:OUTW])
                    nc.scalar.copy(
                        outs[:, OUTW : 2 * OUTW],
                        p2[:, T1OFF : T1OFF + OUTW],
                    )
                for rr in range(2 if dbl == "DMAOUT" else 1):
                    odma.dma_start(out[g * P : (g + 1) * P, :], outs[:, :])

            NP = n_tiles // 2
            for rep in range(passes):
                stL = {}
                stXts = {}
                stLog = {}
                stE = {}
                stSS = {}
                stWb = {}
                stP2 = {}
                stYsb = {}
                stXtp = {}
                stYp = {}
                for i in range(-2, NP + 7):
                    if 0 <= i < NP:
                        stXtp[i] = stageAt(i, stL[i])
                    if 0 <= i + 2 < NP:
                        stL[i + 2] = stageL(i + 2)
                    if 0 <= i - 6 < NP:
                        stageC2(i - 6, stP2.pop(i - 6))
                    if 0 <= i - 5 < NP:
                        stP2[i - 5] = stageC1(
                            i - 5, stL.pop(i - 5), stWb.pop(i - 5)
                        )
                    if 0 <= i - 4 < NP:
                        stWb[i - 4] = stageB2b(
                            i - 4, stE.pop(i - 4), stSS.pop(i - 4)
                        )
                    if 0 <= i - 3 < NP:
                        stE[i - 3] = stageB2a_exp(i - 3, stLog.pop(i - 3))
                    if 0 <= i - 2 < NP:
                        stLog[i - 2] = stageB1x(i - 2, stYsb.pop(i - 2))
                    if 0 <= i - 1 < NP:
                        stYp[i - 1] = stageB1(i - 1, stXts.pop(i - 1))
                    if 0 <= i < NP:
                        stXts[i] = stageAc(i, stXtp.pop(i))
                    if 0 <= i - 1 < NP:
                        stYsb[i - 1] = stageB1c(i - 1, stYp.pop(i - 1))
                    if 0 <= i - 3 < NP:
                        stSS[i - 3] = stageB2a_sadj(i - 3, stE[i - 3])

    return nc


# ---------------------------------------------------------------------------
# Host-side constants + sharding
# ---------------------------------------------------------------------------


def _host_constants(attention_matrix):
    a = np.ascontiguousarray(np.asarray(attention_matrix, dtype=np.float32))
    assert a.shape == (G, D)
    at = a.T  # [768, 130]
    at_r = np.ascontiguousarray(
        at.reshape(DCHUNKS, P, G).transpose(1, 0, 2).reshape(P, DCHUNKS * G)
    ).astype(np.float16)
    ident = np.eye(P, dtype=np.float16)
    iota = np.tile(np.arange(G, dtype=np.float16), (P, 1))
    mask16 = (
        (np.arange(P)[:, None] // SEG) == np.arange(NB)[None, :]
    ).astype(np.float16)
    bagadj = (
        (np.arange(P)[:, None] // SEG) == (np.arange(P)[None, :] // SEG)
    ).astype(np.float16)
    return at_r, ident, iota, mask16, bagadj


def make_in_maps(x, attention_query, attention_matrix):
    at_r, ident, iota, mask16, bagadj = _host_constants(attention_matrix)

    x_pad = np.zeros((N_PAD, D), dtype=np.float16)
    x_pad[:N] = x.astype(np.float16)
    q_pad = np.zeros((N_PAD, 3), dtype=np.float32)
    q_pad[:N] = attention_query.astype(np.float32)

    in_maps = []
    for c in range(NCORES):
        xs = x_pad[c * ROWS_CORE : (c + 1) * ROWS_CORE]
        qs = (
            q_pad[c * ROWS_CORE : (c + 1) * ROWS_CORE]
            .reshape(NT, P, 3)
            .transpose(1, 0, 2)
            .reshape(P, 3 * NT)
        )
        in_maps.append(
            {
                "x": np.ascontiguousarray(xs),
                "q": np.ascontiguousarray(qs),
                "at": at_r,
                "ident": ident,
                "iota": iota,
                "mask16": mask16,
                "bagadj": bagadj,
            }
        )
    return in_maps


def kernel(x, attention_query, scope, attention_matrix):
    x = np.asarray(x)
    attention_query = np.asarray(attention_query)
    assert x.shape == (N, D) and attention_query.shape == (N, 3)

    in_maps = make_in_maps(x, attention_query, attention_matrix)

    nc = build_nc()
    trace = bool(int(os.environ.get("KERNEL_TRACE", "0")))
    res = run_bass_kernel_spmd(
        nc, in_maps, core_ids=list(range(NCORES)), trace=trace
    )
    global LAST_EXEC_TIME_NS, LAST_TRACE_PATH
    LAST_EXEC_TIME_NS = res.exec_time_ns
    if trace:
        print(f"HW exec time: {res.exec_time_ns} ns")
        if res.instructions_and_trace is not None:
            LAST_TRACE_PATH = res.instructions_and_trace[1]
            print("trace:", LAST_TRACE_PATH)

    # per-core out is [(NT/2)*128, 2*288] fp16: pair g, partition p, col
    # k*288 + 48j + 16l + b = out[l, 16(2g+k) + b, 128j + p]
    parts = [
        r["out"]
        .reshape(NT // 2, P, 2, DCHUNKS, 3, NB)
        .transpose(4, 0, 2, 5, 3, 1)
        .reshape(3, BAGS_CORE, D)
        for r in res.results
    ]
    full = np.concatenate(parts, axis=1)[:, :B, :].astype(np.float32)
    return np.ascontiguousarray(full)
